# revision 14
# baseline (speedup 1.0000x reference)
"""Trainium2 Bass kernel for nn_GNN_53145925321329 (GNN message passing).

Key algebraic fact: the reference computes a full [B, N_ENT, D] segment-sum,
but the output only reads segment `entity[0]`:

    out = u * tanh(agg[:, e0, :] @ W0)
    agg[:, e0, :] = sum_{edges e: rows[e]==e0} rel_w[:, values[e]] * entity_emb[cols[e]]

So the only O(E) work is scanning rows == e0 (the memory-bound part, sharded
edge-parallel across the 8 cores per the sharding hint); the ~16 surviving
edges feed a tiny dense matmul chain.

Phase 1 (8 cores): each core scans the uint16 low halves of its E/8 edge
rows (half the HBM bytes of an f32/int32 scan; high-half mismatches are
filtered exactly by the host rescan below) and emits per-(partition, chunk)
match counts. The compare target rides in column 0 of the same tensor, so
the whole phase is one DMA per chunk, two chunks issued in parallel on the
two HWDGE rings (sync + scalar).
Host: resolves exact matched edge positions from the per-chunk counts
(rescans only the few 784-element windows with count>0 against the original
int32 rows — exact for any multiplicity and immune to low-half aliasing),
gathers values/cols/entity_emb rows for those edges.
Phase 2 (1 core): all operands packed into ONE [128, nk*20+28] f32 tensor
(single input DMA, issued first):
  relwT = relT^T@uT; T = rone^T@Emat; aggT = T^T@relwT; l0T = W0^T@aggT;
  out = u*tanh(l0). Operands are fed pre-transposed so no on-device
transposes are needed.
"""

import numpy as np

import concourse.bacc as bacc
import concourse.mybir as mybir
import concourse.tile as tile
from concourse import bass_utils

# Problem shapes (hardcoded per contract)
E = 1_600_000
D = 8
B = 8
R = 12
N_CORES = 8
P = 128
NCH = 4              # chunks per core (2 per HWDGE ring, pipelined reduces)
W = 392              # elements per (partition, chunk)
COLS = NCH * W       # 1568 elements per partition
PER_CORE = P * COLS  # 200_704
E_PAD = PER_CORE * N_CORES

_CACHE = {}


def build_scan():
    """Per-core: count rows_lo16==ent_lo16 per (partition, chunk).

    Input layout [P, 1+COLS] uint16: column 0 is the broadcast compare
    target (so the compiled NEFF is input-independent), columns 1.. are the
    shard's row-index low halves.
    """
    nc = bacc.Bacc("TRN2", debug=False, target_bir_lowering=False,
                   num_devices=N_CORES)
    u16 = mybir.dt.uint16
    f32 = mybir.dt.float32
    rows_in = nc.dram_tensor("rows", [P, 1 + COLS], u16,
                             kind="ExternalInput").ap()
    cnt_out = nc.dram_tensor("cnt", [P, NCH], f32, kind="ExternalOutput").ap()
    with tile.TileContext(nc) as tc:
        with tc.tile_pool(name="sbuf", bufs=1) as pool:
            # Chunk DMAs alternate between the two HWDGE rings (sync issues
            # even chunks, scalar odd) so issue+transfer overlap.
            tiles = []
            for ch in range(NCH):
                hdr = 1 if ch == 0 else 0
                t = pool.tile([P, hdr + W], u16, tag=f"t{ch}")
                eng = nc.sync if ch % 2 == 0 else nc.scalar
                lo_c = 1 + ch * W
                eng.dma_start(t[:], rows_in[:, lo_c - hdr:lo_c + W])
                tiles.append(t)
            cnt_t = pool.tile([P, NCH], f32)
            ent_f = pool.tile([P, 1], f32)
            nc.vector.tensor_copy(ent_f[:], tiles[0][:, :1])
            for ch in range(NCH):
                hdr = 1 if ch == 0 else 0
                m = pool.tile([P, W], u16, tag=f"m{ch}")
                nc.vector.tensor_scalar(
                    out=m[:],
                    in0=tiles[ch][:, hdr:hdr + W],
                    scalar1=ent_f[:, :1],
                    scalar2=0.0,
                    op0=mybir.AluOpType.is_equal,
                    op1=mybir.AluOpType.add,
                    accum_out=cnt_t[:, ch:ch + 1],
                )
            nc.sync.dma_start(cnt_out[:], cnt_t[:])
    nc.compile()
    return nc


KP = 48              # phase-2 edge slots (partition dim of the tail matmul)


def build_phase2(nk: int):
    """Single-core dense tail on the ~K matched edges (K <= nk*KP).

    One packed f32 input [KP, nk*20+28]:
      block k: [k*20, k*20+8)   = Emat_k   (entity_emb rows, [KP, 8])
               [k*20+8, k*20+20)= rone_k   (one-hot of relation, [KP, 12])
      tail (rows 0..7 used):    [nk*20+ 0, nk*20+ 8) = uT   [D, B]
                                [nk*20+ 8, nk*20+20) = relT [D, R]
                                [nk*20+20, nk*20+28) = w0   [D, D]
    """
    nc = bacc.Bacc("TRN2", debug=False, target_bir_lowering=False,
                   num_devices=1)
    f32 = mybir.dt.float32
    ncols = nk * 20 + 28
    pk_in = nc.dram_tensor("pk", [KP, ncols], f32, kind="ExternalInput").ap()
    outT = nc.dram_tensor("outT", [D, B], f32, kind="ExternalOutput").ap()
    tb = nk * 20  # tail base column

    with tile.TileContext(nc) as tc:
        with (
            tc.tile_pool(name="sbuf", bufs=1) as pool,
            tc.tile_pool(name="psum", bufs=1, space="PSUM") as psum,
        ):
            pk = pool.tile([KP, ncols], f32)
            nc.sync.dma_start(pk[:], pk_in[:])
            uT = pk[0:D, tb:tb + 8]
            relT = pk[0:D, tb + 8:tb + 20]
            w0 = pk[0:D, tb + 20:tb + 28]

            # T[r, d] = sum_k onehot(vals)[k, r] * Emat[k, d]
            t_ps = psum.tile([R, D], f32)
            for k in range(nk):
                nc.tensor.matmul(out=t_ps[:],
                                 lhsT=pk[:, k * 20 + 8:k * 20 + 20],
                                 rhs=pk[:, k * 20:k * 20 + 8],
                                 start=(k == 0), stop=(k == nk - 1))
            # relwT[r, b] = sum_d relation_emb[r, d] * u[b, d]
            relw_ps = psum.tile([R, B], f32)
            nc.tensor.matmul(out=relw_ps[:], lhsT=relT, rhs=uT,
                             start=True, stop=True)
            t_sb = pool.tile([R, D], f32)
            nc.vector.tensor_copy(t_sb[:], t_ps[:])
            relw_sb = pool.tile([R, B], f32)
            nc.vector.tensor_copy(relw_sb[:], relw_ps[:])

            # aggT[d, b] = sum_r T[r, d] * relwT[r, b]
            agg_ps = psum.tile([D, B], f32)
            nc.tensor.matmul(out=agg_ps[:], lhsT=t_sb[:], rhs=relw_sb[:],
                             start=True, stop=True)
            agg_sb = pool.tile([D, B], f32)
            nc.vector.tensor_copy(agg_sb[:], agg_ps[:])

            # l0T[dd, b] = sum_d w0[d, dd] * aggT[d, b]
            l0_ps = psum.tile([D, B], f32)
            nc.tensor.matmul(out=l0_ps[:], lhsT=w0, rhs=agg_sb[:],
                             start=True, stop=True)

            rep_sb = pool.tile([D, B], f32)
            nc.scalar.activation(rep_sb[:], l0_ps[:],
                                 mybir.ActivationFunctionType.Tanh)
            out_sb = pool.tile([D, B], f32)
            nc.vector.tensor_mul(out_sb[:], uT, rep_sb[:])
            nc.sync.dma_start(outT[:], out_sb[:])
    nc.compile()
    return nc


def _get(name, builder, *args):
    key = (name,) + args
    if key not in _CACHE:
        _CACHE[key] = builder(*args)
    return _CACHE[key]


def kernel(user, entity, values, indices, user_emb, relation_emb, entity_emb,
           weight_0) -> np.ndarray:
    user = np.asarray(user)
    entity = np.asarray(entity)
    values = np.asarray(values)
    indices = np.asarray(indices)
    user_emb = np.asarray(user_emb, dtype=np.float32)
    relation_emb = np.asarray(relation_emb, dtype=np.float32)
    entity_emb = np.asarray(entity_emb, dtype=np.float32)
    weight_0 = np.asarray(weight_0, dtype=np.float32)

    ent0 = int(entity[0])
    rows = np.asarray(indices[0], dtype=np.int32)

    # ---- Phase 1: sharded uint16 low-half edge scan on 8 cores ----
    rows_pad = np.full(E_PAD, -1, dtype=np.int32)
    rows_pad[:E] = rows
    lo = (rows_pad & 0xFFFF).astype(np.uint16).reshape(N_CORES, P, COLS)
    shards = np.empty((N_CORES, P, 1 + COLS), dtype=np.uint16)
    shards[:, :, 0] = np.uint16(ent0 & 0xFFFF)
    shards[:, :, 1:] = lo

    nc1 = _get("scan", build_scan)
    res1 = bass_utils.run_bass_kernel_spmd(
        nc1,
        [{"rows": np.ascontiguousarray(shards[c])} for c in range(N_CORES)],
        core_ids=list(range(N_CORES)),
    )
    counts = np.stack([r["cnt"] for r in res1.results])  # [N_CORES, P, NCH]

    # ---- Host: resolve exact matched edge ids from per-chunk counts ----
    # (counts may include low-half aliases; the rescan against the original
    # int32 rows filters them exactly.)
    view = rows_pad.reshape(N_CORES, P, NCH, W)
    matched = []
    for c, p, ch in np.argwhere(counts > 0.5):
        for w in np.nonzero(view[c, p, ch] == ent0)[0]:
            matched.append(c * PER_CORE + p * COLS + ch * W + w)
    g = np.array(sorted(matched), dtype=np.int64)

    k_n = len(g)
    nk = max(1, -(-k_n // KP))
    pk = np.zeros((KP, nk * 20 + 28), np.float32)
    if k_n:
        emat = np.zeros((nk * KP, D), np.float32)
        rone = np.zeros((nk * KP, R), np.float32)
        emat[:k_n] = entity_emb[indices[1][g]]
        rone[np.arange(k_n), values[g]] = 1.0
        for k in range(nk):
            pk[:, k * 20:k * 20 + 8] = emat[k * KP:(k + 1) * KP]
            pk[:, k * 20 + 8:k * 20 + 20] = rone[k * KP:(k + 1) * KP]

    # ---- Phase 2: dense tail on one core ----
    u = user_emb[user]  # [B, D]
    tb = nk * 20
    pk[0:D, tb:tb + 8] = u.T
    pk[0:D, tb + 8:tb + 20] = relation_emb.T
    pk[0:D, tb + 20:tb + 28] = weight_0

    nc2 = _get("phase2", build_phase2, nk)
    res2 = bass_utils.run_bass_kernel_spmd(nc2, [{"pk": pk}], core_ids=[0])
    outT = res2.results[0]["outT"]
    return np.ascontiguousarray(outT.T, dtype=np.float32)


# revision 15
# speedup vs baseline: 1.0219x; 1.0219x over previous
"""Trainium2 Bass kernel for nn_GNN_53145925321329 (GNN message passing).

Key algebraic fact: the reference computes a full [B, N_ENT, D] segment-sum,
but the output only reads segment `entity[0]`:

    out = u * tanh(agg[:, e0, :] @ W0)
    agg[:, e0, :] = sum_{edges e: rows[e]==e0} rel_w[:, values[e]] * entity_emb[cols[e]]

So the only O(E) work is scanning rows == e0 (the memory-bound part, sharded
edge-parallel across the 8 cores per the sharding hint); the ~16 surviving
edges feed a tiny dense matmul chain.

Phase 1 (8 cores): each core scans the uint16 low halves of its E/8 edge
rows (half the HBM bytes of an f32/int32 scan; high-half mismatches are
filtered exactly by the host rescan below) and emits per-(partition, chunk)
match counts. The compare target rides in column 0 of the same tensor, so
the whole phase is one DMA per chunk, two chunks issued in parallel on the
two HWDGE rings (sync + scalar).
Host: resolves exact matched edge positions from the per-chunk counts
(rescans only the few 784-element windows with count>0 against the original
int32 rows — exact for any multiplicity and immune to low-half aliasing),
gathers values/cols/entity_emb rows for those edges.
Phase 2 (1 core): all operands packed into ONE [128, nk*20+28] f32 tensor
(single input DMA, issued first):
  relwT = relT^T@uT; T = rone^T@Emat; aggT = T^T@relwT; l0T = W0^T@aggT;
  out = u*tanh(l0). Operands are fed pre-transposed so no on-device
transposes are needed.
"""

import numpy as np

import concourse.bacc as bacc
import concourse.mybir as mybir
import concourse.tile as tile
from concourse import bass_utils

# Problem shapes (hardcoded per contract)
E = 1_600_000
D = 8
B = 8
R = 12
N_CORES = 8
P = 128
NCH = 4              # chunks per core (2 per HWDGE ring, pipelined reduces)
W = 392              # elements per (partition, chunk)
COLS = NCH * W       # 1568 elements per partition
PER_CORE = P * COLS  # 200_704
E_PAD = PER_CORE * N_CORES

_CACHE = {}


def build_scan():
    """Per-core: count rows_lo16==ent_lo16 per (partition, chunk).

    Input layout [P, 1+COLS] uint16: column 0 is the broadcast compare
    target (so the compiled NEFF is input-independent), columns 1.. are the
    shard's row-index low halves.
    """
    nc = bacc.Bacc("TRN2", debug=False, target_bir_lowering=False,
                   num_devices=N_CORES)
    u16 = mybir.dt.uint16
    f32 = mybir.dt.float32
    rows_in = nc.dram_tensor("rows", [P, 1 + COLS], u16,
                             kind="ExternalInput").ap()
    cnt_out = nc.dram_tensor("cnt", [P, NCH], f32, kind="ExternalOutput").ap()
    with tile.TileContext(nc) as tc:
        with tc.tile_pool(name="sbuf", bufs=1) as pool:
            # Chunk DMAs alternate between the two HWDGE rings (sync issues
            # even chunks, scalar odd) so issue+transfer overlap.
            tiles = []
            for ch in range(NCH):
                hdr = 1 if ch == 0 else 0
                t = pool.tile([P, hdr + W], u16, tag=f"t{ch}")
                eng = nc.sync if ch % 2 == 0 else nc.scalar
                lo_c = 1 + ch * W
                eng.dma_start(t[:], rows_in[:, lo_c - hdr:lo_c + W])
                tiles.append(t)
            cnt_t = pool.tile([P, NCH], f32)
            ent_f = pool.tile([P, 1], f32)
            nc.vector.tensor_copy(ent_f[:], tiles[0][:, :1])
            for ch in range(NCH):
                hdr = 1 if ch == 0 else 0
                m = pool.tile([P, W], f32, tag=f"m{ch}")
                nc.vector.tensor_scalar(
                    out=m[:],
                    in0=tiles[ch][:, hdr:hdr + W],
                    scalar1=ent_f[:, :1],
                    scalar2=0.0,
                    op0=mybir.AluOpType.is_equal,
                    op1=mybir.AluOpType.add,
                    accum_out=cnt_t[:, ch:ch + 1],
                )
            nc.sync.dma_start(cnt_out[:], cnt_t[:])
    nc.compile()
    return nc


KP = 48              # phase-2 edge slots (partition dim of the tail matmul)


def build_phase2(nk: int):
    """Single-core dense tail on the ~K matched edges (K <= nk*KP).

    One packed f32 input [KP, nk*20+28]:
      block k: [k*20, k*20+8)   = Emat_k   (entity_emb rows, [KP, 8])
               [k*20+8, k*20+20)= rone_k   (one-hot of relation, [KP, 12])
      tail (rows 0..7 used):    [nk*20+ 0, nk*20+ 8) = uT   [D, B]
                                [nk*20+ 8, nk*20+20) = relT [D, R]
                                [nk*20+20, nk*20+28) = w0   [D, D]
    """
    nc = bacc.Bacc("TRN2", debug=False, target_bir_lowering=False,
                   num_devices=1)
    f32 = mybir.dt.float32
    ncols = nk * 20 + 28
    pk_in = nc.dram_tensor("pk", [KP, ncols], f32, kind="ExternalInput").ap()
    outT = nc.dram_tensor("outT", [D, B], f32, kind="ExternalOutput").ap()
    tb = nk * 20  # tail base column

    with tile.TileContext(nc) as tc:
        with (
            tc.tile_pool(name="sbuf", bufs=1) as pool,
            tc.tile_pool(name="psum", bufs=1, space="PSUM") as psum,
        ):
            pk = pool.tile([KP, ncols], f32)
            nc.sync.dma_start(pk[:], pk_in[:])
            uT = pk[0:D, tb:tb + 8]
            relT = pk[0:D, tb + 8:tb + 20]
            w0 = pk[0:D, tb + 20:tb + 28]

            # T[r, d] = sum_k onehot(vals)[k, r] * Emat[k, d]
            t_ps = psum.tile([R, D], f32)
            for k in range(nk):
                nc.tensor.matmul(out=t_ps[:],
                                 lhsT=pk[:, k * 20 + 8:k * 20 + 20],
                                 rhs=pk[:, k * 20:k * 20 + 8],
                                 start=(k == 0), stop=(k == nk - 1))
            # relwT[r, b] = sum_d relation_emb[r, d] * u[b, d]
            relw_ps = psum.tile([R, B], f32)
            nc.tensor.matmul(out=relw_ps[:], lhsT=relT, rhs=uT,
                             start=True, stop=True)
            t_sb = pool.tile([R, D], f32)
            nc.vector.tensor_copy(t_sb[:], t_ps[:])
            relw_sb = pool.tile([R, B], f32)
            nc.vector.tensor_copy(relw_sb[:], relw_ps[:])

            # aggT[d, b] = sum_r T[r, d] * relwT[r, b]
            agg_ps = psum.tile([D, B], f32)
            nc.tensor.matmul(out=agg_ps[:], lhsT=t_sb[:], rhs=relw_sb[:],
                             start=True, stop=True)
            agg_sb = pool.tile([D, B], f32)
            nc.vector.tensor_copy(agg_sb[:], agg_ps[:])

            # l0T[dd, b] = sum_d w0[d, dd] * aggT[d, b]
            l0_ps = psum.tile([D, B], f32)
            nc.tensor.matmul(out=l0_ps[:], lhsT=w0, rhs=agg_sb[:],
                             start=True, stop=True)

            rep_sb = pool.tile([D, B], f32)
            nc.scalar.activation(rep_sb[:], l0_ps[:],
                                 mybir.ActivationFunctionType.Tanh)
            out_sb = pool.tile([D, B], f32)
            nc.vector.tensor_mul(out_sb[:], uT, rep_sb[:])
            nc.sync.dma_start(outT[:], out_sb[:])
    nc.compile()
    return nc


def _get(name, builder, *args):
    key = (name,) + args
    if key not in _CACHE:
        _CACHE[key] = builder(*args)
    return _CACHE[key]


def kernel(user, entity, values, indices, user_emb, relation_emb, entity_emb,
           weight_0) -> np.ndarray:
    user = np.asarray(user)
    entity = np.asarray(entity)
    values = np.asarray(values)
    indices = np.asarray(indices)
    user_emb = np.asarray(user_emb, dtype=np.float32)
    relation_emb = np.asarray(relation_emb, dtype=np.float32)
    entity_emb = np.asarray(entity_emb, dtype=np.float32)
    weight_0 = np.asarray(weight_0, dtype=np.float32)

    ent0 = int(entity[0])
    rows = np.asarray(indices[0], dtype=np.int32)

    # ---- Phase 1: sharded uint16 low-half edge scan on 8 cores ----
    rows_pad = np.full(E_PAD, -1, dtype=np.int32)
    rows_pad[:E] = rows
    lo = (rows_pad & 0xFFFF).astype(np.uint16).reshape(N_CORES, P, COLS)
    shards = np.empty((N_CORES, P, 1 + COLS), dtype=np.uint16)
    shards[:, :, 0] = np.uint16(ent0 & 0xFFFF)
    shards[:, :, 1:] = lo

    nc1 = _get("scan", build_scan)
    res1 = bass_utils.run_bass_kernel_spmd(
        nc1,
        [{"rows": np.ascontiguousarray(shards[c])} for c in range(N_CORES)],
        core_ids=list(range(N_CORES)),
    )
    counts = np.stack([r["cnt"] for r in res1.results])  # [N_CORES, P, NCH]

    # ---- Host: resolve exact matched edge ids from per-chunk counts ----
    # (counts may include low-half aliases; the rescan against the original
    # int32 rows filters them exactly.)
    view = rows_pad.reshape(N_CORES, P, NCH, W)
    matched = []
    for c, p, ch in np.argwhere(counts > 0.5):
        for w in np.nonzero(view[c, p, ch] == ent0)[0]:
            matched.append(c * PER_CORE + p * COLS + ch * W + w)
    g = np.array(sorted(matched), dtype=np.int64)

    k_n = len(g)
    nk = max(1, -(-k_n // KP))
    pk = np.zeros((KP, nk * 20 + 28), np.float32)
    if k_n:
        emat = np.zeros((nk * KP, D), np.float32)
        rone = np.zeros((nk * KP, R), np.float32)
        emat[:k_n] = entity_emb[indices[1][g]]
        rone[np.arange(k_n), values[g]] = 1.0
        for k in range(nk):
            pk[:, k * 20:k * 20 + 8] = emat[k * KP:(k + 1) * KP]
            pk[:, k * 20 + 8:k * 20 + 20] = rone[k * KP:(k + 1) * KP]

    # ---- Phase 2: dense tail on one core ----
    u = user_emb[user]  # [B, D]
    tb = nk * 20
    pk[0:D, tb:tb + 8] = u.T
    pk[0:D, tb + 8:tb + 20] = relation_emb.T
    pk[0:D, tb + 20:tb + 28] = weight_0

    nc2 = _get("phase2", build_phase2, nk)
    res2 = bass_utils.run_bass_kernel_spmd(nc2, [{"pk": pk}], core_ids=[0])
    outT = res2.results[0]["outT"]
    return np.ascontiguousarray(outT.T, dtype=np.float32)


# revision 16
# speedup vs baseline: 1.0327x; 1.0106x over previous
"""Trainium2 Bass kernel for nn_GNN_53145925321329 (GNN message passing).

Key algebraic fact: the reference computes a full [B, N_ENT, D] segment-sum,
but the output only reads segment `entity[0]`:

    out = u * tanh(agg[:, e0, :] @ W0)
    agg[:, e0, :] = sum_{edges e: rows[e]==e0} rel_w[:, values[e]] * entity_emb[cols[e]]

So the only O(E) work is scanning rows == e0 (the memory-bound part, sharded
edge-parallel across the 8 cores per the sharding hint); the ~16 surviving
edges feed a tiny dense matmul chain.

Phase 1 (8 cores): each core scans the uint16 low halves of its E/8 edge
rows (half the HBM bytes of an f32/int32 scan; high-half mismatches are
filtered exactly by the host rescan below) and emits per-(partition, chunk)
match counts. The compare target rides in column 0 of the same tensor, so
the whole phase is one DMA per chunk, two chunks issued in parallel on the
two HWDGE rings (sync + scalar).
Host: resolves exact matched edge positions from the per-chunk counts
(rescans only the few 784-element windows with count>0 against the original
int32 rows — exact for any multiplicity and immune to low-half aliasing),
gathers values/cols/entity_emb rows for those edges.
Phase 2 (1 core): all operands packed into ONE [128, nk*20+28] f32 tensor
(single input DMA, issued first):
  relwT = relT^T@uT; T = rone^T@Emat; aggT = T^T@relwT; l0T = W0^T@aggT;
  out = u*tanh(l0). Operands are fed pre-transposed so no on-device
transposes are needed.
"""

import numpy as np

import concourse.bacc as bacc
import concourse.mybir as mybir
import concourse.tile as tile
from concourse import bass_utils

# Problem shapes (hardcoded per contract)
E = 1_600_000
D = 8
B = 8
R = 12
N_CORES = 8
P = 128
NCH = 4              # chunks per core (2 per HWDGE ring, pipelined reduces)
W = 392              # elements per (partition, chunk)
COLS = NCH * W       # 1568 elements per partition
PER_CORE = P * COLS  # 200_704
E_PAD = PER_CORE * N_CORES

_CACHE = {}


def build_scan():
    """Per-core: count rows_lo16==ent_lo16 per (partition, chunk).

    Input layout [P, 1+COLS] uint16: column 0 is the broadcast compare
    target (so the compiled NEFF is input-independent), columns 1.. are the
    shard's row-index low halves.
    """
    nc = bacc.Bacc("TRN2", debug=False, target_bir_lowering=False,
                   num_devices=N_CORES)
    u16 = mybir.dt.uint16
    f32 = mybir.dt.float32
    rows_in = nc.dram_tensor("rows", [P, 1 + COLS], u16,
                             kind="ExternalInput").ap()
    cnt_out = nc.dram_tensor("cnt", [P, NCH], f32, kind="ExternalOutput").ap()
    with tile.TileContext(nc) as tc:
        with tc.tile_pool(name="sbuf", bufs=1) as pool:
            # Chunk DMAs alternate between the two HWDGE rings (sync issues
            # even chunks, scalar odd) so issue+transfer overlap.
            tiles = []
            for ch in range(NCH):
                hdr = 1 if ch == 0 else 0
                t = pool.tile([P, hdr + W], u16, tag=f"t{ch}")
                eng = (nc.sync, nc.scalar, nc.gpsimd, nc.sync)[ch]
                lo_c = 1 + ch * W
                eng.dma_start(t[:], rows_in[:, lo_c - hdr:lo_c + W])
                tiles.append(t)
            cnt_t = pool.tile([P, NCH], f32)
            ent_f = pool.tile([P, 1], f32)
            nc.vector.tensor_copy(ent_f[:], tiles[0][:, :1])
            for ch in range(NCH):
                hdr = 1 if ch == 0 else 0
                m = pool.tile([P, W], f32, tag=f"m{ch}")
                nc.vector.tensor_scalar(
                    out=m[:],
                    in0=tiles[ch][:, hdr:hdr + W],
                    scalar1=ent_f[:, :1],
                    scalar2=0.0,
                    op0=mybir.AluOpType.is_equal,
                    op1=mybir.AluOpType.add,
                    accum_out=cnt_t[:, ch:ch + 1],
                )
            nc.sync.dma_start(cnt_out[:], cnt_t[:])
    nc.compile()
    return nc


KP = 48              # phase-2 edge slots (partition dim of the tail matmul)


def build_phase2(nk: int):
    """Single-core dense tail on the ~K matched edges (K <= nk*KP).

    One packed f32 input [KP, nk*20+28]:
      block k: [k*20, k*20+8)   = Emat_k   (entity_emb rows, [KP, 8])
               [k*20+8, k*20+20)= rone_k   (one-hot of relation, [KP, 12])
      tail (rows 0..7 used):    [nk*20+ 0, nk*20+ 8) = uT   [D, B]
                                [nk*20+ 8, nk*20+20) = relT [D, R]
                                [nk*20+20, nk*20+28) = w0   [D, D]
    """
    nc = bacc.Bacc("TRN2", debug=False, target_bir_lowering=False,
                   num_devices=1)
    f32 = mybir.dt.float32
    ncols = nk * 20 + 28
    pk_in = nc.dram_tensor("pk", [KP, ncols], f32, kind="ExternalInput").ap()
    outT = nc.dram_tensor("outT", [D, B], f32, kind="ExternalOutput").ap()
    tb = nk * 20  # tail base column

    with tile.TileContext(nc) as tc:
        with (
            tc.tile_pool(name="sbuf", bufs=1) as pool,
            tc.tile_pool(name="psum", bufs=1, space="PSUM") as psum,
        ):
            pk = pool.tile([KP, ncols], f32)
            nc.sync.dma_start(pk[:], pk_in[:])
            uT = pk[0:D, tb:tb + 8]
            relT = pk[0:D, tb + 8:tb + 20]
            w0 = pk[0:D, tb + 20:tb + 28]

            # T[r, d] = sum_k onehot(vals)[k, r] * Emat[k, d]
            t_ps = psum.tile([R, D], f32)
            for k in range(nk):
                nc.tensor.matmul(out=t_ps[:],
                                 lhsT=pk[:, k * 20 + 8:k * 20 + 20],
                                 rhs=pk[:, k * 20:k * 20 + 8],
                                 start=(k == 0), stop=(k == nk - 1))
            # relwT[r, b] = sum_d relation_emb[r, d] * u[b, d]
            relw_ps = psum.tile([R, B], f32)
            nc.tensor.matmul(out=relw_ps[:], lhsT=relT, rhs=uT,
                             start=True, stop=True)
            t_sb = pool.tile([R, D], f32)
            nc.vector.tensor_copy(t_sb[:], t_ps[:])
            relw_sb = pool.tile([R, B], f32)
            nc.vector.tensor_copy(relw_sb[:], relw_ps[:])

            # aggT[d, b] = sum_r T[r, d] * relwT[r, b]
            agg_ps = psum.tile([D, B], f32)
            nc.tensor.matmul(out=agg_ps[:], lhsT=t_sb[:], rhs=relw_sb[:],
                             start=True, stop=True)
            agg_sb = pool.tile([D, B], f32)
            nc.vector.tensor_copy(agg_sb[:], agg_ps[:])

            # l0T[dd, b] = sum_d w0[d, dd] * aggT[d, b]
            l0_ps = psum.tile([D, B], f32)
            nc.tensor.matmul(out=l0_ps[:], lhsT=w0, rhs=agg_sb[:],
                             start=True, stop=True)

            rep_sb = pool.tile([D, B], f32)
            nc.scalar.activation(rep_sb[:], l0_ps[:],
                                 mybir.ActivationFunctionType.Tanh)
            out_sb = pool.tile([D, B], f32)
            nc.vector.tensor_mul(out_sb[:], uT, rep_sb[:])
            nc.sync.dma_start(outT[:], out_sb[:])
    nc.compile()
    return nc


def _get(name, builder, *args):
    key = (name,) + args
    if key not in _CACHE:
        _CACHE[key] = builder(*args)
    return _CACHE[key]


def kernel(user, entity, values, indices, user_emb, relation_emb, entity_emb,
           weight_0) -> np.ndarray:
    user = np.asarray(user)
    entity = np.asarray(entity)
    values = np.asarray(values)
    indices = np.asarray(indices)
    user_emb = np.asarray(user_emb, dtype=np.float32)
    relation_emb = np.asarray(relation_emb, dtype=np.float32)
    entity_emb = np.asarray(entity_emb, dtype=np.float32)
    weight_0 = np.asarray(weight_0, dtype=np.float32)

    ent0 = int(entity[0])
    rows = np.asarray(indices[0], dtype=np.int32)

    # ---- Phase 1: sharded uint16 low-half edge scan on 8 cores ----
    rows_pad = np.full(E_PAD, -1, dtype=np.int32)
    rows_pad[:E] = rows
    lo = (rows_pad & 0xFFFF).astype(np.uint16).reshape(N_CORES, P, COLS)
    shards = np.empty((N_CORES, P, 1 + COLS), dtype=np.uint16)
    shards[:, :, 0] = np.uint16(ent0 & 0xFFFF)
    shards[:, :, 1:] = lo

    nc1 = _get("scan", build_scan)
    res1 = bass_utils.run_bass_kernel_spmd(
        nc1,
        [{"rows": np.ascontiguousarray(shards[c])} for c in range(N_CORES)],
        core_ids=list(range(N_CORES)),
    )
    counts = np.stack([r["cnt"] for r in res1.results])  # [N_CORES, P, NCH]

    # ---- Host: resolve exact matched edge ids from per-chunk counts ----
    # (counts may include low-half aliases; the rescan against the original
    # int32 rows filters them exactly.)
    view = rows_pad.reshape(N_CORES, P, NCH, W)
    matched = []
    for c, p, ch in np.argwhere(counts > 0.5):
        for w in np.nonzero(view[c, p, ch] == ent0)[0]:
            matched.append(c * PER_CORE + p * COLS + ch * W + w)
    g = np.array(sorted(matched), dtype=np.int64)

    k_n = len(g)
    nk = max(1, -(-k_n // KP))
    pk = np.zeros((KP, nk * 20 + 28), np.float32)
    if k_n:
        emat = np.zeros((nk * KP, D), np.float32)
        rone = np.zeros((nk * KP, R), np.float32)
        emat[:k_n] = entity_emb[indices[1][g]]
        rone[np.arange(k_n), values[g]] = 1.0
        for k in range(nk):
            pk[:, k * 20:k * 20 + 8] = emat[k * KP:(k + 1) * KP]
            pk[:, k * 20 + 8:k * 20 + 20] = rone[k * KP:(k + 1) * KP]

    # ---- Phase 2: dense tail on one core ----
    u = user_emb[user]  # [B, D]
    tb = nk * 20
    pk[0:D, tb:tb + 8] = u.T
    pk[0:D, tb + 8:tb + 20] = relation_emb.T
    pk[0:D, tb + 20:tb + 28] = weight_0

    nc2 = _get("phase2", build_phase2, nk)
    res2 = bass_utils.run_bass_kernel_spmd(nc2, [{"pk": pk}], core_ids=[0])
    outT = res2.results[0]["outT"]
    return np.ascontiguousarray(outT.T, dtype=np.float32)


# revision 17
# speedup vs baseline: 1.0465x; 1.0133x over previous
"""Trainium2 Bass kernel for nn_GNN_53145925321329 (GNN message passing).

Key algebraic fact: the reference computes a full [B, N_ENT, D] segment-sum,
but the output only reads segment `entity[0]`:

    out = u * tanh(agg[:, e0, :] @ W0)
    agg[:, e0, :] = sum_{edges e: rows[e]==e0} rel_w[:, values[e]] * entity_emb[cols[e]]

So the only O(E) work is scanning rows == e0 (the memory-bound part, sharded
edge-parallel across the 8 cores per the sharding hint); the ~16 surviving
edges feed a tiny dense matmul chain.

Phase 1 (8 cores): each core scans the uint16 low halves of its E/8 edge
rows (half the HBM bytes of an f32/int32 scan; high-half mismatches are
filtered exactly by the host rescan below) and emits per-(partition, chunk)
match counts. The compare target rides in column 0 of the same tensor, so
the whole phase is one DMA per chunk, two chunks issued in parallel on the
two HWDGE rings (sync + scalar).
Host: resolves exact matched edge positions from the per-chunk counts
(rescans only the few 784-element windows with count>0 against the original
int32 rows — exact for any multiplicity and immune to low-half aliasing),
gathers values/cols/entity_emb rows for those edges.
Phase 2 (1 core): all operands packed into ONE [128, nk*20+28] f32 tensor
(single input DMA, issued first):
  relwT = relT^T@uT; T = rone^T@Emat; aggT = T^T@relwT; l0T = W0^T@aggT;
  out = u*tanh(l0). Operands are fed pre-transposed so no on-device
transposes are needed.
"""

import numpy as np

import concourse.bacc as bacc
import concourse.mybir as mybir
import concourse.tile as tile
from concourse import bass_utils

# Problem shapes (hardcoded per contract)
E = 1_600_000
D = 8
B = 8
R = 12
N_CORES = 8
P = 128
NCH = 4              # chunks per core (2 per HWDGE ring, pipelined reduces)
W = 392              # elements per (partition, chunk)
COLS = NCH * W       # 1568 elements per partition
PER_CORE = P * COLS  # 200_704
E_PAD = PER_CORE * N_CORES

_CACHE = {}


def build_scan():
    """Per-core: count rows_lo16==ent_lo16 per (partition, chunk).

    Input layout [P, 1+COLS] uint16: column 0 is the broadcast compare
    target (so the compiled NEFF is input-independent), columns 1.. are the
    shard's row-index low halves.
    """
    nc = bacc.Bacc("TRN2", debug=False, target_bir_lowering=False,
                   num_devices=N_CORES)
    u16 = mybir.dt.uint16
    f32 = mybir.dt.float32
    rows_in = nc.dram_tensor("rows", [P, 1 + COLS], u16,
                             kind="ExternalInput").ap()
    cnt_out = nc.dram_tensor("cnt", [P, NCH], f32, kind="ExternalOutput").ap()
    with tile.TileContext(nc) as tc:
        with tc.tile_pool(name="sbuf", bufs=1) as pool:
            # Chunk DMAs alternate between the two HWDGE rings (sync issues
            # even chunks, scalar odd) so issue+transfer overlap.
            tiles = []
            for ch in range(NCH):
                hdr = 1 if ch == 0 else 0
                t = pool.tile([P, hdr + W], u16, tag=f"t{ch}")
                eng = (nc.sync, nc.scalar, nc.gpsimd, nc.sync)[ch]
                lo_c = 1 + ch * W
                eng.dma_start(t[:], rows_in[:, lo_c - hdr:lo_c + W])
                tiles.append(t)
            cnt_t = pool.tile([P, NCH], f32)
            ent_f = pool.tile([P, 1], f32)
            nc.vector.tensor_copy(ent_f[:], tiles[0][:, :1])
            for ch in range(NCH):
                hdr = 1 if ch == 0 else 0
                m = pool.tile([P, W], f32, tag=f"m{ch}")
                nc.vector.tensor_scalar(
                    out=m[:],
                    in0=tiles[ch][:, hdr:hdr + W],
                    scalar1=ent_f[:, :1],
                    scalar2=0.0,
                    op0=mybir.AluOpType.is_equal,
                    op1=mybir.AluOpType.add,
                    accum_out=cnt_t[:, ch:ch + 1],
                )
            nc.sync.dma_start(cnt_out[:], cnt_t[:])
    nc.compile()
    return nc


KP = 16              # phase-2 edge slots (partition dim of the tail matmul)


def build_phase2(nk: int):
    """Single-core dense tail on the ~K matched edges (K <= nk*KP).

    One packed f32 input [KP, nk*20+28]:
      block k: [k*20, k*20+8)   = Emat_k   (entity_emb rows, [KP, 8])
               [k*20+8, k*20+20)= rone_k   (one-hot of relation, [KP, 12])
      tail (rows 0..7 used):    [nk*20+ 0, nk*20+ 8) = uT   [D, B]
                                [nk*20+ 8, nk*20+20) = relT [D, R]
                                [nk*20+20, nk*20+28) = w0   [D, D]
    """
    nc = bacc.Bacc("TRN2", debug=False, target_bir_lowering=False,
                   num_devices=1)
    f32 = mybir.dt.float32
    ncols = nk * 20 + 28
    pk_in = nc.dram_tensor("pk", [KP, ncols], f32, kind="ExternalInput").ap()
    outT = nc.dram_tensor("outT", [D, B], f32, kind="ExternalOutput").ap()
    tb = nk * 20  # tail base column

    with tile.TileContext(nc) as tc:
        with (
            tc.tile_pool(name="sbuf", bufs=1) as pool,
            tc.tile_pool(name="psum", bufs=1, space="PSUM") as psum,
        ):
            pk = pool.tile([KP, ncols], f32)
            nc.sync.dma_start(pk[:], pk_in[:])
            uT = pk[0:D, tb:tb + 8]
            relT = pk[0:D, tb + 8:tb + 20]
            w0 = pk[0:D, tb + 20:tb + 28]

            # T[r, d] = sum_k onehot(vals)[k, r] * Emat[k, d]
            t_ps = psum.tile([R, D], f32)
            for k in range(nk):
                nc.tensor.matmul(out=t_ps[:],
                                 lhsT=pk[:, k * 20 + 8:k * 20 + 20],
                                 rhs=pk[:, k * 20:k * 20 + 8],
                                 start=(k == 0), stop=(k == nk - 1))
            # relwT[r, b] = sum_d relation_emb[r, d] * u[b, d]
            relw_ps = psum.tile([R, B], f32)
            nc.tensor.matmul(out=relw_ps[:], lhsT=relT, rhs=uT,
                             start=True, stop=True)
            t_sb = pool.tile([R, D], f32)
            nc.vector.tensor_copy(t_sb[:], t_ps[:])
            relw_sb = pool.tile([R, B], f32)
            nc.vector.tensor_copy(relw_sb[:], relw_ps[:])

            # aggT[d, b] = sum_r T[r, d] * relwT[r, b]
            agg_ps = psum.tile([D, B], f32)
            nc.tensor.matmul(out=agg_ps[:], lhsT=t_sb[:], rhs=relw_sb[:],
                             start=True, stop=True)
            agg_sb = pool.tile([D, B], f32)
            nc.vector.tensor_copy(agg_sb[:], agg_ps[:])

            # l0T[dd, b] = sum_d w0[d, dd] * aggT[d, b]
            l0_ps = psum.tile([D, B], f32)
            nc.tensor.matmul(out=l0_ps[:], lhsT=w0, rhs=agg_sb[:],
                             start=True, stop=True)

            rep_sb = pool.tile([D, B], f32)
            nc.scalar.activation(rep_sb[:], l0_ps[:],
                                 mybir.ActivationFunctionType.Tanh)
            out_sb = pool.tile([D, B], f32)
            nc.vector.tensor_mul(out_sb[:], uT, rep_sb[:])
            nc.sync.dma_start(outT[:], out_sb[:])
    nc.compile()
    return nc


def _get(name, builder, *args):
    key = (name,) + args
    if key not in _CACHE:
        _CACHE[key] = builder(*args)
    return _CACHE[key]


def kernel(user, entity, values, indices, user_emb, relation_emb, entity_emb,
           weight_0) -> np.ndarray:
    user = np.asarray(user)
    entity = np.asarray(entity)
    values = np.asarray(values)
    indices = np.asarray(indices)
    user_emb = np.asarray(user_emb, dtype=np.float32)
    relation_emb = np.asarray(relation_emb, dtype=np.float32)
    entity_emb = np.asarray(entity_emb, dtype=np.float32)
    weight_0 = np.asarray(weight_0, dtype=np.float32)

    ent0 = int(entity[0])
    rows = np.asarray(indices[0], dtype=np.int32)

    # ---- Phase 1: sharded uint16 low-half edge scan on 8 cores ----
    rows_pad = np.full(E_PAD, -1, dtype=np.int32)
    rows_pad[:E] = rows
    lo = (rows_pad & 0xFFFF).astype(np.uint16).reshape(N_CORES, P, COLS)
    shards = np.empty((N_CORES, P, 1 + COLS), dtype=np.uint16)
    shards[:, :, 0] = np.uint16(ent0 & 0xFFFF)
    shards[:, :, 1:] = lo

    nc1 = _get("scan", build_scan)
    res1 = bass_utils.run_bass_kernel_spmd(
        nc1,
        [{"rows": np.ascontiguousarray(shards[c])} for c in range(N_CORES)],
        core_ids=list(range(N_CORES)),
    )
    counts = np.stack([r["cnt"] for r in res1.results])  # [N_CORES, P, NCH]

    # ---- Host: resolve exact matched edge ids from per-chunk counts ----
    # (counts may include low-half aliases; the rescan against the original
    # int32 rows filters them exactly.)
    view = rows_pad.reshape(N_CORES, P, NCH, W)
    matched = []
    for c, p, ch in np.argwhere(counts > 0.5):
        for w in np.nonzero(view[c, p, ch] == ent0)[0]:
            matched.append(c * PER_CORE + p * COLS + ch * W + w)
    g = np.array(sorted(matched), dtype=np.int64)

    k_n = len(g)
    nk = max(1, -(-k_n // KP))
    pk = np.zeros((KP, nk * 20 + 28), np.float32)
    if k_n:
        emat = np.zeros((nk * KP, D), np.float32)
        rone = np.zeros((nk * KP, R), np.float32)
        emat[:k_n] = entity_emb[indices[1][g]]
        rone[np.arange(k_n), values[g]] = 1.0
        for k in range(nk):
            pk[:, k * 20:k * 20 + 8] = emat[k * KP:(k + 1) * KP]
            pk[:, k * 20 + 8:k * 20 + 20] = rone[k * KP:(k + 1) * KP]

    # ---- Phase 2: dense tail on one core ----
    u = user_emb[user]  # [B, D]
    tb = nk * 20
    pk[0:D, tb:tb + 8] = u.T
    pk[0:D, tb + 8:tb + 20] = relation_emb.T
    pk[0:D, tb + 20:tb + 28] = weight_0

    nc2 = _get("phase2", build_phase2, nk)
    res2 = bass_utils.run_bass_kernel_spmd(nc2, [{"pk": pk}], core_ids=[0])
    outT = res2.results[0]["outT"]
    return np.ascontiguousarray(outT.T, dtype=np.float32)
